# revision 47
# baseline (speedup 1.0000x reference)
"""AgentImputer Trainium2 kernel.

Contract: kernel(**inputs) takes the FULL unsharded inputs (as produced by
reference.setup_inputs()) and returns the FULL output [64, 40, 2] float32.

Strategy: data-parallel over batch B=64 across 8 NeuronCores (8 batches /
core -> 320 folded LSTM rows per core). The 128-step TimeLSTM runs
feature-major ([hid, row] tiles); categorical embeddings fold into the
input matmul via one-hot rows; biases fold into matmuls via a constant-1
state row. The recurrent loop is software-pipelined as TWO independent
column groups (rows 0:160 / 160:320) so the serial h->gates->c->h chain of
one group overlaps engine work of the other. All elementwise state math is
bf16 (DVE 2x packed mode); sigmoid over all 4 gates of a group is a single
ACT instruction; (ts-1) is host-precomputed and DMA-broadcast per step; the
per-graph GCN (shared edge_index) is dense [40,40] mean-aggregation matmuls.
"""

import sys

import numpy as np

sys.path.insert(0, "/opt/trn_rl_repo")

# ---------------------------------------------------------------- constants
B, W, N, F_IN = 64, 128, 40, 66
HID = 100
NUM_CONT = 64
NCLS_POS, NCLS_TEAM = 16, 9
EMB_POS, EMB_TEAM = 4, 3
NCORES = 8
BL = B // NCORES          # 8 local batch elems per core
R = BL * N                # 320 rows per core; row j = 40*b_local + n
RGS = [160, 160]          # pipeline column groups
OH_P0 = 66                # one-hot pos cols [66:83)
OH_T0 = 83                # one-hot team cols [83:100) (entries 10..16 pad)
XC = 100                  # xs tile feature columns
G4 = 4 * HID
TB = 8                    # timestep block for X prefetch


# ---------------------------------------------------------------- host prep
def _host_weights(inputs):
    import ml_dtypes
    bf = ml_dtypes.bfloat16
    f32 = np.float32
    Uall_w = np.asarray(inputs["Uall_w"], f32)       # [400, 71]
    Uall_b = np.asarray(inputs["Uall_b"], f32)       # [400]
    Wall_w = np.asarray(inputs["Wall_w"], f32)       # [400, 100]
    Wall_b = np.asarray(inputs["Wall_b"], f32)       # [400]
    Wd_w = np.asarray(inputs["Wd_w"], f32)           # [100, 100]
    Wd_b = np.asarray(inputs["Wd_b"], f32)           # [100]
    lin_w = np.asarray(inputs["lin_w"], f32)         # [100, 100]
    lin_b = np.asarray(inputs["lin_b"], f32)         # [100]
    emb_pos = np.asarray(inputs["emb_pos"], f32)     # [16, 4]
    emb_team = np.asarray(inputs["emb_team"], f32)   # [9, 3]
    edge_index = np.asarray(inputs["edge_index"]).astype(np.int64)  # [2, E]

    # Input-side weights [100, 400]: rows 0:64 continuous features; rows
    # 64,65 (raw categorical codes riding along in the transposed tile) get
    # zero weights; rows 66:83 / 83:93 are one-hot rows with the embedding
    # tables pre-multiplied in (code 0 == missing -> zero row); 93:100 pad.
    WxT = np.zeros((XC, G4), f32)
    WxT[0:NUM_CONT] = Uall_w[:, 0:NUM_CONT].T
    pad_pos = np.vstack([np.zeros((1, EMB_POS), f32), emb_pos])    # [17, 4]
    pad_team = np.vstack([np.zeros((1, EMB_TEAM), f32), emb_team])  # [10, 3]
    WxT[OH_P0:OH_T0] = pad_pos @ Uall_w[:, NUM_CONT:NUM_CONT + EMB_POS].T
    WxT[OH_T0:OH_T0 + NCLS_TEAM + 1] = (
        pad_team @ Uall_w[:, NUM_CONT + EMB_POS:].T
    )

    # h-side weights with the full gate bias folded in as an extra row
    # (state tiles carry a constant-1 row at partition HID).
    WallT = np.concatenate([Wall_w.T, (Wall_b + Uall_b)[None, :]], 0)  # [101, 400]
    WdT = np.concatenate([Wd_w.T, Wd_b[None, :]], 0)                   # [101, 100]
    linT = np.concatenate([lin_w.T, lin_b[None, :]], 0)                # [101, 100]

    # Mean-aggregation matrix: M[s, d] = count(s->d) / max(deg(d), 1)
    src, dst = edge_index[0], edge_index[1]
    cnt = np.zeros((N, N), f32)
    np.add.at(cnt, (src, dst), 1.0)
    deg = np.maximum(cnt.sum(axis=0), 1.0)
    Mmat = cnt / deg[None, :]

    # iota rows for the merged one-hot compare: [0..16 | 0..9, -1 x7],
    # replicated for each timestep of an 8-step block
    iota2 = np.concatenate([
        np.arange(NCLS_POS + 1, dtype=f32),
        np.concatenate([np.arange(NCLS_TEAM + 1, dtype=f32),
                        -np.ones(17 - (NCLS_TEAM + 1), f32)]),
    ])
    iota2b = np.tile(iota2, TB)  # [TB*34]

    # gate-chunk order (f, i, ct, o): lets the kernel pair (f,i) with
    # (cadj,ct) in one strided DVE multiply
    gperm = np.concatenate([np.arange(100), np.arange(100, 200),
                            np.arange(300, 400), np.arange(200, 300)])
    WxT = WxT[:, gperm]
    WallT = WallT[:, gperm]

    return {
        "WxT": WxT.astype(bf),
        "WallT": WallT.astype(bf),
        "WdT": WdT.astype(bf),
        "linT": linT.astype(bf),
        "Mmat3": np.ascontiguousarray(
            np.kron(np.eye(3, dtype=f32), Mmat), f32),  # [120,120] blockdiag
        "s1l": np.ascontiguousarray(np.asarray(inputs["sage1_l"], f32).T),   # [100, 64]
        "s1r": np.ascontiguousarray(np.asarray(inputs["sage1_r"], f32).T),   # [100, 64]
        "s1b": np.ascontiguousarray(np.asarray(inputs["sage1_lb"], f32)[:, None]),  # [64, 1]
        "s2l": np.ascontiguousarray(np.asarray(inputs["sage2_l"], f32).T),   # [64, 32]
        "s2r": np.ascontiguousarray(np.asarray(inputs["sage2_r"], f32).T),   # [64, 32]
        "s2b": np.ascontiguousarray(np.asarray(inputs["sage2_lb"], f32)[:, None]),  # [32, 1]
        "ow": np.ascontiguousarray(np.asarray(inputs["out_w"], f32).T),      # [32, 2]
        "ob": np.ascontiguousarray(np.asarray(inputs["out_b"], f32)[:, None]),      # [2, 1]
        "iota2b": np.tile(iota2b, (120, 1)).astype(bf),                      # [120, TB*34]
        "hcinit": np.concatenate(
            [np.zeros((HID, R), f32), np.ones((1, R), f32)], 0
        ).astype(bf),  # [101, R]: zero state + constant-1 bias row
        "ident": np.eye(128, dtype=f32),
        "identb": np.eye(128, dtype=bf),
    }


# ---------------------------------------------------------------- device IR
def build_module(Wsteps=W):
    import concourse.bass as bass
    import concourse.tile as tile
    from concourse import bacc, mybir

    f32 = mybir.dt.float32
    f32r = mybir.dt.float32r
    bf16 = mybir.dt.bfloat16
    AF = mybir.ActivationFunctionType
    EQ = mybir.AluOpType.is_equal
    PSUM = bass.MemorySpace.PSUM

    def r(ap):
        return ap.bitcast(f32r)

    nc = bacc.Bacc(
        "TRN2", target_bir_lowering=False, debug=False, num_devices=NCORES
    )

    X_in = nc.declare_dram_parameter("X", [BL, W, N, F_IN], bf16, isOutput=False)
    # host-precomputed (ts-1), replicated across 100 partitions: [100, W, R]
    tsm1_in = nc.declare_dram_parameter("tsm1", [HID, W, R], bf16, isOutput=False)
    w_in = {}
    bf16_params = {"WxT", "WallT", "WdT", "linT", "iota2b", "identb", "hcinit"}
    for name, shape in [
        ("WxT", [XC, G4]), ("WallT", [HID + 1, G4]), ("WdT", [HID + 1, HID]),
        ("linT", [HID + 1, HID]), ("Mmat3", [3 * N, 3 * N]),
        ("s1l", [HID, 64]), ("s1r", [HID, 64]), ("s1b", [64, 1]),
        ("s2l", [64, 32]), ("s2r", [64, 32]), ("s2b", [32, 1]),
        ("ow", [32, 2]), ("ob", [2, 1]),
        ("iota2b", [120, TB * 34]), ("hcinit", [HID + 1, R]),
        ("ident", [128, 128]), ("identb", [128, 128]),
    ]:
        w_in[name] = nc.declare_dram_parameter(
            name, shape, bf16 if name in bf16_params else f32r, isOutput=False
        )
    # device-natural layout [k, b, n]; host transposes to [b, n, k]
    out_ext = nc.declare_dram_parameter("out", [2, BL, N], f32, isOutput=True)

    GSL = [slice(0, RGS[0]), slice(RGS[0], R)]

    with tile.TileContext(nc) as tc:
        with (
            tc.tile_pool(name="consts", bufs=1) as consts,
            tc.tile_pool(name="state", bufs=1) as state,
        ):
            # ---- load constants / weights
            wt = {}
            qs = [nc.gpsimd, nc.sync]
            for qi, (name, ext) in enumerate(w_in.items()):
                wt[name] = consts.tile(
                    list(ext.shape), ext.dtype, tag=name, name=name
                )
                qs[qi % 2].dma_start(out=wt[name][:], in_=ext[:])

            # ---- persistent state: h/c feature-major with const-1 bias row
            hT = state.tile([HID + 1, R], bf16, tag="hT")
            cT = state.tile([HID + 1, R], bf16, tag="cT")
            nc.gpsimd.dma_start(out=hT[:], in_=w_in["hcinit"][:])
            nc.gpsimd.dma_start(out=cT[:], in_=w_in["hcinit"][:])

            nodesT = state.tile([HID, R], f32r, tag="nodesT")

            Xnb = X_in.rearrange("b t n f -> b n t f")

            with (
                tc.tile_pool(name="xs", bufs=2) as xs_pool,
                tc.tile_pool(name="xf", bufs=2) as xf_pool,
                tc.tile_pool(name="tsb", bufs=2) as tsb_pool,
                tc.tile_pool(name="sg", bufs=2) as sg_pool,
                tc.tile_pool(name="work", bufs=2) as work,
                tc.tile_pool(name="pga", bufs=1, space=PSUM) as pga_pool,
                tc.tile_pool(name="pgb", bufs=1, space=PSUM) as pgb_pool,
                tc.tile_pool(name="pd", bufs=1, space=PSUM) as pd_pool,
                tc.tile_pool(name="pxf", bufs=2, space=PSUM) as pxf_pool,
            ):
                TRIPLES = [(0, 3), (3, 3), (6, 2)]

                def load_block(t0):
                    """DMA one TB-step X block (one-hots emitted separately)."""
                    tiles = []
                    for k, (b0, nb) in enumerate(TRIPLES):
                        xt = xs_pool.tile([120, TB, XC], bf16,
                                          tag=f"xs{k}", name=f"xs{k}")
                        for i in range(nb):
                            nc.sync.dma_start(
                                out=xt[N * i:N * (i + 1), :, 0:F_IN],
                                in_=Xnb[b0 + i, :, t0:t0 + TB, :],
                            )
                        tiles.append(xt)
                    return tiles

                def emit_onehot(xtiles, k):
                    # merged one-hot for triple k: both categorical cols,
                    # all TB steps, all stacked graphs in one op
                    rows = N * TRIPLES[k][1]
                    xt = xtiles[k]
                    nc.vector.tensor_tensor(
                        out=xt[:rows, :, OH_P0:XC].rearrange(
                            "p t (g k) -> p t g k", k=17
                        ),
                        in0=wt["iota2b"][0:rows, :].rearrange(
                            "p (t g k) -> p t g k", t=TB, k=17
                        ),
                        in1=xt[
                            :rows, :, NUM_CONT:NUM_CONT + 2
                        ].to_broadcast([rows, TB, 2, 17]),
                        op=EQ,
                    )

                def emit_trans(xtiles, tl):
                    """PE transposes -> pxf psum; returns psum tile."""
                    pxf = pxf_pool.tile([XC, R], bf16, tag="pxf")
                    for k, (b0, nb) in enumerate(TRIPLES):
                        rows = N * nb
                        nc.tensor.transpose(
                            pxf[:, 120 * k:120 * k + rows],
                            xtiles[k][:rows, tl, :],
                            wt["identb"][:rows, :rows],
                        )
                    return pxf

                def emit_tsb(t):
                    tsb = tsb_pool.tile([HID, R], bf16, tag="tsb")
                    nc.sync.dma_start(out=tsb[:], in_=tsm1_in[:, t, :])
                    return tsb

                def emit_xmm(pg, gi, g, xfT):
                    rg = RGS[gi]
                    sl = slice((g % 2) * rg, (g % 2) * rg + rg)
                    nc.tensor.matmul(
                        pg[:, g // 2, sl],
                        wt["WxT"][:, HID * g:HID * (g + 1)],
                        xfT[:, GSL[gi]], start=(g % 2 == 0), stop=False,
                    )

                def emit_wd(pdn, gi):
                    # two half-width Wd matmuls share one psum bank: the A
                    # half's start arms the bank, B's half closes the group;
                    # each fires as soon as its own c' half lands.
                    nc.tensor.matmul(pdn[:, GSL[gi]], wt["WdT"][:],
                                     cT[:, GSL[gi]],
                                     start=(gi == 0), stop=(gi == 1))

                # ---- prologue: block 0. Steps 0-1 are DMA'd first as a thin
                # slice so compute starts while the rest of the block loads.
                xcur = []
                for k, (b0, nb) in enumerate(TRIPLES):
                    xt = xs_pool.tile([120, TB, XC], bf16,
                                      tag=f"xs{k}", name=f"xs{k}")
                    for i in range(nb):
                        nc.sync.dma_start(
                            out=xt[N * i:N * (i + 1), 0:2, 0:F_IN],
                            in_=Xnb[b0 + i, :, 0:2, :],
                        )
                    rows = N * nb
                    nc.vector.tensor_tensor(
                        out=xt[:rows, 0:2, OH_P0:XC].rearrange(
                            "p t (g k) -> p t g k", k=17
                        ),
                        in0=wt["iota2b"][0:rows, 0:2 * 34].rearrange(
                            "p (t g k) -> p t g k", t=2, k=17
                        ),
                        in1=xt[
                            :rows, 0:2, NUM_CONT:NUM_CONT + 2
                        ].to_broadcast([rows, 2, 2, 17]),
                        op=EQ,
                    )
                    xcur.append(xt)
                for k, (b0, nb) in enumerate(TRIPLES):
                    xt = xcur[k]
                    for i in range(nb):
                        nc.sync.dma_start(
                            out=xt[N * i:N * (i + 1), 2:TB, 0:F_IN],
                            in_=Xnb[b0 + i, :, 2:TB, :],
                        )
                    rows = N * nb
                    nc.vector.tensor_tensor(
                        out=xt[:rows, 2:TB, OH_P0:XC].rearrange(
                            "p t (g k) -> p t g k", k=17
                        ),
                        in0=wt["iota2b"][0:rows, 0:(TB - 2) * 34].rearrange(
                            "p (t g k) -> p t g k", t=TB - 2, k=17
                        ),
                        in1=xt[
                            :rows, 2:TB, NUM_CONT:NUM_CONT + 2
                        ].to_broadcast([rows, TB - 2, 2, 17]),
                        op=EQ,
                    )
                pxf0 = emit_trans(xcur, 0)
                xfT = xf_pool.tile([XC, R], bf16, tag="xfT")
                nc.vector.tensor_scalar_add(xfT[:], pxf0[:], 0.0)
                pgA = pga_pool.tile([HID, 2, 512], f32, tag="pgA", name="pgA")
                pgB = pgb_pool.tile([HID, 2, 512], f32, tag="pgB", name="pgB")
                pgrp = [pgA, pgB]
                for gi in range(2):
                    for g in range(4):
                        emit_xmm(pgrp[gi], gi, g, xfT)
                tsb = emit_tsb(0)
                pds = [pd_pool.tile([HID, 512], f32, tag="pd0", name="pd0")]
                emit_wd(pds[0], 0)
                emit_wd(pds[0], 1)
                xnext_fresh = False

                for t in range(Wsteps):
                    tl = t % TB
                    last = t == Wsteps - 1

                    t1 = work.tile([HID, R], bf16, tag="t1")
                    cs1 = work.tile([HID, R], bf16, tag="cs1")
                    tnc = work.tile([HID, R], bf16, tag="tnc")
                    pp = work.tile([HID, 2, R], bf16, tag="pp")
                    # sg slots: 0=f, 1=cadj, 2=i, 3=ct, 4=unused, 5=o
                    sg = sg_pool.tile([HID, 6, R], bf16, tag="sg")

                    # ---- c path (off the critical h-chain); cadj lands in
                    # sg slot 1, adjacent to the gates. t1/cadj split per
                    # group, A first, so cadj_A is ready before sigA ends.
                    nc.scalar.activation(cs1[:], pds[0][:, 0:R], AF.Tanh)
                    for gi in range(2):
                        gsl = GSL[gi]
                        nc.vector.tensor_mul(t1[:, gsl], cs1[:, gsl],
                                             tsb[:, gsl])
                        nc.vector.tensor_add(sg[:, 1, gsl], cT[0:HID, gsl],
                                             t1[:, gsl])

                    # mid-block prefetch of the next X block
                    if tl == 4 and t + 4 < Wsteps:
                        xnext = load_block(t + 4)
                        xnext_fresh = True

                    tsb_n = None if last else emit_tsb(t + 1)

                    # next step's xfT: transposes go behind hmm_A on PE; the
                    # psum->sbuf copy sits early in the DVE stream (it parks
                    # until the transposes land, while later DVE ops bypass).
                    if not last:
                        if tl == TB - 1:
                            xcur = xnext
                        pxf = emit_trans(xcur, (t + 1) % TB)

                    for gi in range(2):
                        gsl = GSL[gi]
                        pg = pgrp[gi]

                        # h-side accumulate onto the x-side psum
                        rg = RGS[gi]
                        for g in range(4):
                            sl = slice((g % 2) * rg, (g % 2) * rg + rg)
                            nc.tensor.matmul(
                                pg[:, g // 2, sl],
                                wt["WallT"][:, HID * g:HID * (g + 1)],
                                hT[:, gsl], start=False, stop=(g % 2 == 1),
                            )

                        # one sigmoid instruction for all 4 gates of group;
                        # psum gate order (f,i | ct,o) -> sg slots (0,2|3,5)
                        nc.scalar.activation(
                            sg[:, 0:6, gsl].rearrange(
                                "p (b r) c -> p b r c", b=2)[:, :, ::2, :],
                            pg[:, :, 0:2 * rg].rearrange(
                                "p b (s c) -> p b s c", c=rg
                            ),
                            AF.Sigmoid,
                        )

                        # state update: c' = f*cadj + i*ct via one paired
                        # multiply over slots (0,2)x(1,3) then one add
                        pair = sg[:, 0:4, gsl].rearrange(
                            "p (a b) c -> p a b c", b=2)
                        nc.vector.tensor_mul(pp[:, :, gsl], pair[:, :, 0, :],
                                             pair[:, :, 1, :])
                        nc.vector.tensor_add(cT[0:HID, gsl], pp[:, 0, gsl],
                                             pp[:, 1, gsl])

                        # next step's x-side matmuls reuse this group's freed
                        # pg banks right after its sigmoid; Wd(t+1) parks
                        # in the PE wait queue and fires the moment c'_B lands.
                        if not last:
                            emit_wd(pds[0], gi)
                            if gi == 1:
                                # the xfT copy sits AFTER c'_B in the DVE
                                # program so the B-tail never queues behind
                                # it; both groups' x-matmuls follow it
                                xfT = xf_pool.tile([XC, R], bf16, tag="xfT")
                                nc.vector.tensor_scalar_add(xfT[:], pxf[:],
                                                            0.0)
                                for gj in range(2):
                                    for g in range(4):
                                        emit_xmm(pgrp[gj], gj, g, xfT)

                    # ---- step tails: h' = o*tanh(c') for both groups
                    for gi in range(2):
                        gsl = GSL[gi]
                        nc.scalar.activation(tnc[:, gsl], cT[0:HID, gsl],
                                             AF.Tanh)
                        nc.vector.tensor_mul(hT[0:HID, gsl], sg[:, 5, gsl],
                                             tnc[:, gsl])

                    # one-hot expansion for the prefetched block rides in the
                    # DVE lull at step tails (one triple per step)
                    if xnext_fresh and tl in (4, 5, 6):
                        emit_onehot(xnext, tl - 4)
                        if tl == 6:
                            xnext_fresh = False

                    tsb = tsb_n

                # ---- output linear: nodes = relu(lin @ h + lb)
                pl = pd_pool.tile([HID, 512], f32, tag="pd0", name="pl")
                nc.tensor.matmul(pl[:, 0:R], wt["linT"][:], hT[:],
                                 start=True, stop=True)
                nc.scalar.activation(nodesT[:], pl[:, 0:R], AF.Relu)

            # ---------------- GCN: two SAGE layers + output proj
            with (
                tc.tile_pool(name="gc", bufs=2) as gc,
                tc.tile_pool(name="gcs", bufs=1) as gcs,
                tc.tile_pool(name="gp", bufs=2, space=PSUM) as gp,
                tc.tile_pool(name="gp1", bufs=1, space=PSUM) as gp1,
            ):
                def mean_agg(srcT, hid):
                    """srcT: [hid, R] feature-major -> aggT [hid, R].

                    Batches 3 graphs per transpose/matmul using the
                    block-diagonal [120,120] mean matrix: [g0|g1|g2] @ M3 =
                    [g0@M | g1@M | g2@M].
                    """
                    aggT = gcs.tile([hid, R], f32r, tag=f"agg{hid}", name="aggT")
                    for b0, nb in [(0, 3), (3, 3), (6, 2)]:
                        rows = N * nb
                        cols = srcT[:, N * b0:N * (b0 + nb)]   # [hid, 120]
                        ptr = gp.tile([3 * N, 128], f32, tag="ptr")
                        nc.tensor.transpose(
                            r(ptr[0:rows, 0:hid]), cols, wt["ident"][:hid, :hid]
                        )
                        nbm = gc.tile([3 * N, 128], f32r, tag="nbm")
                        nc.any.tensor_copy(out=nbm[0:rows, 0:hid],
                                           in_=ptr[0:rows, 0:hid])
                        pa = gp.tile([128, 3 * N], f32, tag="pa")
                        nc.tensor.matmul(
                            pa[0:hid, 0:rows], nbm[0:rows, 0:hid],
                            wt["Mmat3"][0:rows, 0:rows],
                            start=True, stop=True,
                        )
                        nc.any.tensor_copy(
                            out=aggT[:, N * b0:N * (b0 + nb)],
                            in_=pa[0:hid, 0:rows]
                        )
                    return aggT

                agg1 = mean_agg(nodesT, HID)
                pg1 = gp1.tile([64, R], f32, tag="pg1")
                nc.tensor.matmul(pg1, wt["s1l"][:], agg1[:], start=True, stop=False)
                nc.tensor.matmul(pg1, wt["s1r"][:], nodesT[:], start=False, stop=True)
                g1T = gcs.tile([64, R], f32r, tag="g1T")
                nc.scalar.activation(g1T[:], pg1, AF.Relu, bias=wt["s1b"][:].bitcast(f32))

                agg2 = mean_agg(g1T, 64)
                pg2 = gp1.tile([32, R], f32, tag="pg2")
                nc.tensor.matmul(pg2, wt["s2l"][:], agg2[:], start=True, stop=False)
                nc.tensor.matmul(pg2, wt["s2r"][:], g1T[:], start=False, stop=True)
                g2T = gcs.tile([32, R], f32r, tag="g2T")
                nc.scalar.activation(g2T[:], pg2, AF.Relu, bias=wt["s2b"][:].bitcast(f32))

                po = gp1.tile([2, R], f32, tag="po")
                nc.tensor.matmul(po, wt["ow"][:], g2T[:], start=True, stop=True)
                oT = gcs.tile([2, R], f32, tag="oT")
                nc.scalar.activation(oT[:], po, AF.Relu, bias=wt["ob"][:].bitcast(f32))

                nc.sync.dma_start(
                    out=out_ext.rearrange("k b n -> k (b n)"), in_=oT[:]
                )

    nc.compile()
    return nc


# ---------------------------------------------------------------- execution
_CACHE = {}


def _get_module():
    if "nc" not in _CACHE:
        _CACHE["nc"] = build_module()
    return _CACHE["nc"]


def make_in_maps(inputs):
    f32 = np.float32
    import ml_dtypes
    bf = ml_dtypes.bfloat16
    X = np.ascontiguousarray(np.asarray(inputs["X"], f32).astype(bf))
    ts = np.asarray(inputs["ts_list"], f32)
    wts = _host_weights(inputs)
    in_maps = []
    for c in range(NCORES):
        tsl = ts[c * BL:(c + 1) * BL]                       # [BL, W, N]
        tsm1 = (tsl.transpose(1, 0, 2).reshape(W, R) - 1.0).astype(bf)
        tsm1_rep = np.ascontiguousarray(
            np.broadcast_to(tsm1[None], (HID, W, R))
        )
        m = {"X": X[c * BL:(c + 1) * BL], "tsm1": tsm1_rep}
        m.update(wts)
        in_maps.append(m)
    return in_maps


def kernel(**inputs) -> np.ndarray:
    from concourse.bass_utils import run_bass_kernel_spmd

    nc = _get_module()
    in_maps = make_in_maps(inputs)
    res = run_bass_kernel_spmd(nc, in_maps, list(range(NCORES)))
    outs = [
        np.transpose(res.results[c]["out"], (1, 2, 0)) for c in range(NCORES)
    ]
    return np.ascontiguousarray(np.concatenate(outs, axis=0).astype(np.float32))


# revision 48
# speedup vs baseline: 1.0056x; 1.0056x over previous
"""AgentImputer Trainium2 kernel.

Contract: kernel(**inputs) takes the FULL unsharded inputs (as produced by
reference.setup_inputs()) and returns the FULL output [64, 40, 2] float32.

Strategy: data-parallel over batch B=64 across 8 NeuronCores (8 batches /
core -> 320 folded LSTM rows per core). The 128-step TimeLSTM runs
feature-major ([hid, row] tiles); categorical embeddings fold into the
input matmul via one-hot rows; biases fold into matmuls via a constant-1
state row. The recurrent loop is software-pipelined as TWO independent
column groups (rows 0:160 / 160:320) so the serial h->gates->c->h chain of
one group overlaps engine work of the other. All elementwise state math is
bf16 (DVE 2x packed mode); sigmoid over all 4 gates of a group is a single
ACT instruction; (ts-1) is host-precomputed and DMA-broadcast per step; the
per-graph GCN (shared edge_index) is dense [40,40] mean-aggregation matmuls.
"""

import sys

import numpy as np

sys.path.insert(0, "/opt/trn_rl_repo")

# ---------------------------------------------------------------- constants
B, W, N, F_IN = 64, 128, 40, 66
HID = 100
NUM_CONT = 64
NCLS_POS, NCLS_TEAM = 16, 9
EMB_POS, EMB_TEAM = 4, 3
NCORES = 8
BL = B // NCORES          # 8 local batch elems per core
R = BL * N                # 320 rows per core; row j = 40*b_local + n
RGS = [160, 160]          # pipeline column groups
OH_P0 = 66                # one-hot pos cols [66:83)
OH_T0 = 83                # one-hot team cols [83:100) (entries 10..16 pad)
XC = 100                  # xs tile feature columns
G4 = 4 * HID
TB = 8                    # timestep block for X prefetch


# ---------------------------------------------------------------- host prep
def _host_weights(inputs):
    import ml_dtypes
    bf = ml_dtypes.bfloat16
    f32 = np.float32
    Uall_w = np.asarray(inputs["Uall_w"], f32)       # [400, 71]
    Uall_b = np.asarray(inputs["Uall_b"], f32)       # [400]
    Wall_w = np.asarray(inputs["Wall_w"], f32)       # [400, 100]
    Wall_b = np.asarray(inputs["Wall_b"], f32)       # [400]
    Wd_w = np.asarray(inputs["Wd_w"], f32)           # [100, 100]
    Wd_b = np.asarray(inputs["Wd_b"], f32)           # [100]
    lin_w = np.asarray(inputs["lin_w"], f32)         # [100, 100]
    lin_b = np.asarray(inputs["lin_b"], f32)         # [100]
    emb_pos = np.asarray(inputs["emb_pos"], f32)     # [16, 4]
    emb_team = np.asarray(inputs["emb_team"], f32)   # [9, 3]
    edge_index = np.asarray(inputs["edge_index"]).astype(np.int64)  # [2, E]

    # Input-side weights [100, 400]: rows 0:64 continuous features; rows
    # 64,65 (raw categorical codes riding along in the transposed tile) get
    # zero weights; rows 66:83 / 83:93 are one-hot rows with the embedding
    # tables pre-multiplied in (code 0 == missing -> zero row); 93:100 pad.
    WxT = np.zeros((XC, G4), f32)
    WxT[0:NUM_CONT] = Uall_w[:, 0:NUM_CONT].T
    pad_pos = np.vstack([np.zeros((1, EMB_POS), f32), emb_pos])    # [17, 4]
    pad_team = np.vstack([np.zeros((1, EMB_TEAM), f32), emb_team])  # [10, 3]
    WxT[OH_P0:OH_T0] = pad_pos @ Uall_w[:, NUM_CONT:NUM_CONT + EMB_POS].T
    WxT[OH_T0:OH_T0 + NCLS_TEAM + 1] = (
        pad_team @ Uall_w[:, NUM_CONT + EMB_POS:].T
    )

    # h-side weights with the full gate bias folded in as an extra row
    # (state tiles carry a constant-1 row at partition HID).
    WallT = np.concatenate([Wall_w.T, (Wall_b + Uall_b)[None, :]], 0)  # [101, 400]
    WdT = np.concatenate([Wd_w.T, Wd_b[None, :]], 0)                   # [101, 100]
    linT = np.concatenate([lin_w.T, lin_b[None, :]], 0)                # [101, 100]

    # Mean-aggregation matrix: M[s, d] = count(s->d) / max(deg(d), 1)
    src, dst = edge_index[0], edge_index[1]
    cnt = np.zeros((N, N), f32)
    np.add.at(cnt, (src, dst), 1.0)
    deg = np.maximum(cnt.sum(axis=0), 1.0)
    Mmat = cnt / deg[None, :]

    # iota rows for the merged one-hot compare: [0..16 | 0..9, -1 x7],
    # replicated for each timestep of an 8-step block
    iota2 = np.concatenate([
        np.arange(NCLS_POS + 1, dtype=f32),
        np.concatenate([np.arange(NCLS_TEAM + 1, dtype=f32),
                        -np.ones(17 - (NCLS_TEAM + 1), f32)]),
    ])
    iota2b = np.tile(iota2, TB)  # [TB*34]

    # gate-chunk order (f, i, ct, o): lets the kernel pair (f,i) with
    # (cadj,ct) in one strided DVE multiply
    gperm = np.concatenate([np.arange(100), np.arange(100, 200),
                            np.arange(300, 400), np.arange(200, 300)])
    WxT = WxT[:, gperm]
    WallT = WallT[:, gperm]

    return {
        "WxT": WxT.astype(bf),
        "WallT": WallT.astype(bf),
        "WdT": WdT.astype(bf),
        "linT": linT.astype(bf),
        "Mmat3": np.ascontiguousarray(
            np.kron(np.eye(3, dtype=f32), Mmat), f32),  # [120,120] blockdiag
        "s1l": np.ascontiguousarray(np.asarray(inputs["sage1_l"], f32).T),   # [100, 64]
        "s1r": np.ascontiguousarray(np.asarray(inputs["sage1_r"], f32).T),   # [100, 64]
        "s1b": np.ascontiguousarray(np.asarray(inputs["sage1_lb"], f32)[:, None]),  # [64, 1]
        "s2l": np.ascontiguousarray(np.asarray(inputs["sage2_l"], f32).T),   # [64, 32]
        "s2r": np.ascontiguousarray(np.asarray(inputs["sage2_r"], f32).T),   # [64, 32]
        "s2b": np.ascontiguousarray(np.asarray(inputs["sage2_lb"], f32)[:, None]),  # [32, 1]
        "ow": np.ascontiguousarray(np.asarray(inputs["out_w"], f32).T),      # [32, 2]
        "ob": np.ascontiguousarray(np.asarray(inputs["out_b"], f32)[:, None]),      # [2, 1]
        "iota2b": np.tile(iota2b, (120, 1)).astype(bf),                      # [120, TB*34]
        "hcinit": np.concatenate(
            [np.zeros((HID, R), f32), np.ones((1, R), f32)], 0
        ).astype(bf),  # [101, R]: zero state + constant-1 bias row
        "ident": np.eye(128, dtype=f32),
        "identb": np.eye(128, dtype=bf),
    }


# ---------------------------------------------------------------- device IR
def build_module(Wsteps=W):
    import concourse.bass as bass
    import concourse.tile as tile
    from concourse import bacc, mybir

    f32 = mybir.dt.float32
    f32r = mybir.dt.float32r
    bf16 = mybir.dt.bfloat16
    AF = mybir.ActivationFunctionType
    EQ = mybir.AluOpType.is_equal
    PSUM = bass.MemorySpace.PSUM

    def r(ap):
        return ap.bitcast(f32r)

    nc = bacc.Bacc(
        "TRN2", target_bir_lowering=False, debug=False, num_devices=NCORES
    )

    X_in = nc.declare_dram_parameter("X", [BL, W, N, F_IN], bf16, isOutput=False)
    # host-precomputed (ts-1), replicated across 100 partitions: [100, W, R]
    tsm1_in = nc.declare_dram_parameter("tsm1", [HID, W, R], bf16, isOutput=False)
    w_in = {}
    bf16_params = {"WxT", "WallT", "WdT", "linT", "iota2b", "identb", "hcinit"}
    for name, shape in [
        ("WxT", [XC, G4]), ("WallT", [HID + 1, G4]), ("WdT", [HID + 1, HID]),
        ("linT", [HID + 1, HID]), ("Mmat3", [3 * N, 3 * N]),
        ("s1l", [HID, 64]), ("s1r", [HID, 64]), ("s1b", [64, 1]),
        ("s2l", [64, 32]), ("s2r", [64, 32]), ("s2b", [32, 1]),
        ("ow", [32, 2]), ("ob", [2, 1]),
        ("iota2b", [120, TB * 34]), ("hcinit", [HID + 1, R]),
        ("ident", [128, 128]), ("identb", [128, 128]),
    ]:
        w_in[name] = nc.declare_dram_parameter(
            name, shape, bf16 if name in bf16_params else f32r, isOutput=False
        )
    # device-natural layout [k, b, n]; host transposes to [b, n, k]
    out_ext = nc.declare_dram_parameter("out", [2, BL, N], f32, isOutput=True)

    GSL = [slice(0, RGS[0]), slice(RGS[0], R)]

    with tile.TileContext(nc) as tc:
        with (
            tc.tile_pool(name="consts", bufs=1) as consts,
            tc.tile_pool(name="state", bufs=1) as state,
        ):
            # ---- load constants / weights
            wt = {}
            qs = [nc.gpsimd, nc.sync]
            for qi, (name, ext) in enumerate(w_in.items()):
                wt[name] = consts.tile(
                    list(ext.shape), ext.dtype, tag=name, name=name
                )
                qs[qi % 2].dma_start(out=wt[name][:], in_=ext[:])

            # ---- persistent state: h/c feature-major with const-1 bias row
            hT = state.tile([HID + 1, R], bf16, tag="hT")
            cT = state.tile([HID + 1, R], bf16, tag="cT")
            nc.gpsimd.dma_start(out=hT[:], in_=w_in["hcinit"][:])
            nc.gpsimd.dma_start(out=cT[:], in_=w_in["hcinit"][:])

            nodesT = state.tile([HID, R], f32r, tag="nodesT")

            Xnb = X_in.rearrange("b t n f -> b n t f")

            with (
                tc.tile_pool(name="xs", bufs=2) as xs_pool,
                tc.tile_pool(name="xf", bufs=2) as xf_pool,
                tc.tile_pool(name="tsb", bufs=2) as tsb_pool,
                tc.tile_pool(name="sg", bufs=2) as sg_pool,
                tc.tile_pool(name="work", bufs=2) as work,
                tc.tile_pool(name="pga", bufs=1, space=PSUM) as pga_pool,
                tc.tile_pool(name="pgb", bufs=1, space=PSUM) as pgb_pool,
                tc.tile_pool(name="pd", bufs=1, space=PSUM) as pd_pool,
                tc.tile_pool(name="pxf", bufs=2, space=PSUM) as pxf_pool,
            ):
                TRIPLES = [(0, 3), (3, 3), (6, 2)]

                def load_block(t0):
                    """DMA one TB-step X block (one-hots emitted separately)."""
                    tiles = []
                    for k, (b0, nb) in enumerate(TRIPLES):
                        xt = xs_pool.tile([120, TB, XC], bf16,
                                          tag=f"xs{k}", name=f"xs{k}")
                        for i in range(nb):
                            nc.sync.dma_start(
                                out=xt[N * i:N * (i + 1), :, 0:F_IN],
                                in_=Xnb[b0 + i, :, t0:t0 + TB, :],
                            )
                        tiles.append(xt)
                    return tiles

                def emit_onehot(xtiles, k):
                    # merged one-hot for triple k: both categorical cols,
                    # all TB steps, all stacked graphs in one op
                    rows = N * TRIPLES[k][1]
                    xt = xtiles[k]
                    nc.vector.tensor_tensor(
                        out=xt[:rows, :, OH_P0:XC].rearrange(
                            "p t (g k) -> p t g k", k=17
                        ),
                        in0=wt["iota2b"][0:rows, :].rearrange(
                            "p (t g k) -> p t g k", t=TB, k=17
                        ),
                        in1=xt[
                            :rows, :, NUM_CONT:NUM_CONT + 2
                        ].to_broadcast([rows, TB, 2, 17]),
                        op=EQ,
                    )

                def emit_trans(xtiles, tl):
                    """PE transposes -> pxf psum; returns psum tile."""
                    pxf = pxf_pool.tile([XC, R], bf16, tag="pxf")
                    for k, (b0, nb) in enumerate(TRIPLES):
                        rows = N * nb
                        nc.tensor.transpose(
                            pxf[:, 120 * k:120 * k + rows],
                            xtiles[k][:rows, tl, :],
                            wt["identb"][:rows, :rows],
                        )
                    return pxf

                def emit_tsb(t):
                    tsb = tsb_pool.tile([HID, R], bf16, tag="tsb")
                    nc.sync.dma_start(out=tsb[:], in_=tsm1_in[:, t, :])
                    return tsb

                def emit_xmm(pg, gi, g, xfT):
                    rg = RGS[gi]
                    sl = slice((g % 2) * rg, (g % 2) * rg + rg)
                    nc.tensor.matmul(
                        pg[:, g // 2, sl],
                        wt["WxT"][:, HID * g:HID * (g + 1)],
                        xfT[:, GSL[gi]], start=(g % 2 == 0), stop=False,
                    )

                def emit_wd(pdn, gi):
                    # two half-width Wd matmuls share one psum bank: the A
                    # half's start arms the bank, B's half closes the group;
                    # each fires as soon as its own c' half lands.
                    nc.tensor.matmul(pdn[:, GSL[gi]], wt["WdT"][:],
                                     cT[:, GSL[gi]],
                                     start=(gi == 0), stop=(gi == 1))

                # ---- prologue: block 0. Steps 0-1 are DMA'd first as a thin
                # slice so compute starts while the rest of the block loads.
                xcur = []
                for k, (b0, nb) in enumerate(TRIPLES):
                    xt = xs_pool.tile([120, TB, XC], bf16,
                                      tag=f"xs{k}", name=f"xs{k}")
                    for i in range(nb):
                        nc.sync.dma_start(
                            out=xt[N * i:N * (i + 1), 0:2, 0:F_IN],
                            in_=Xnb[b0 + i, :, 0:2, :],
                        )
                    rows = N * nb
                    nc.vector.tensor_tensor(
                        out=xt[:rows, 0:2, OH_P0:XC].rearrange(
                            "p t (g k) -> p t g k", k=17
                        ),
                        in0=wt["iota2b"][0:rows, 0:2 * 34].rearrange(
                            "p (t g k) -> p t g k", t=2, k=17
                        ),
                        in1=xt[
                            :rows, 0:2, NUM_CONT:NUM_CONT + 2
                        ].to_broadcast([rows, 2, 2, 17]),
                        op=EQ,
                    )
                    xcur.append(xt)
                for k, (b0, nb) in enumerate(TRIPLES):
                    xt = xcur[k]
                    for i in range(nb):
                        nc.sync.dma_start(
                            out=xt[N * i:N * (i + 1), 2:TB, 0:F_IN],
                            in_=Xnb[b0 + i, :, 2:TB, :],
                        )
                    rows = N * nb
                    nc.vector.tensor_tensor(
                        out=xt[:rows, 2:TB, OH_P0:XC].rearrange(
                            "p t (g k) -> p t g k", k=17
                        ),
                        in0=wt["iota2b"][0:rows, 0:(TB - 2) * 34].rearrange(
                            "p (t g k) -> p t g k", t=TB - 2, k=17
                        ),
                        in1=xt[
                            :rows, 2:TB, NUM_CONT:NUM_CONT + 2
                        ].to_broadcast([rows, TB - 2, 2, 17]),
                        op=EQ,
                    )
                pxf0 = emit_trans(xcur, 0)
                xfT = xf_pool.tile([XC, R], bf16, tag="xfT")
                nc.vector.tensor_scalar_add(xfT[:], pxf0[:], 0.0)
                pgA = pga_pool.tile([HID, 2, 512], f32, tag="pgA", name="pgA")
                pgB = pgb_pool.tile([HID, 2, 512], f32, tag="pgB", name="pgB")
                pgrp = [pgA, pgB]
                for gi in range(2):
                    for g in range(4):
                        emit_xmm(pgrp[gi], gi, g, xfT)
                tsb = emit_tsb(0)
                pds = [pd_pool.tile([HID, 512], f32, tag="pd0", name="pd0")]
                emit_wd(pds[0], 0)
                emit_wd(pds[0], 1)
                xnext_fresh = False

                for t in range(Wsteps):
                    tl = t % TB
                    last = t == Wsteps - 1

                    t1 = work.tile([HID, R], bf16, tag="t1")
                    cs1 = work.tile([HID, R], bf16, tag="cs1")
                    tnc = work.tile([HID, R], bf16, tag="tnc")
                    pp = work.tile([HID, 2, R], bf16, tag="pp")
                    # sg slots: 0=f, 1=cadj, 2=i, 3=ct, 4=unused, 5=o
                    sg = sg_pool.tile([HID, 6, R], bf16, tag="sg")

                    # ---- c path (off the critical h-chain); cadj lands in
                    # sg slot 1, adjacent to the gates. t1/cadj split per
                    # group, A first, so cadj_A is ready before sigA ends.
                    nc.scalar.activation(cs1[:], pds[0][:, 0:R], AF.Tanh)
                    for gi in range(2):
                        gsl = GSL[gi]
                        nc.vector.tensor_mul(t1[:, gsl], cs1[:, gsl],
                                             tsb[:, gsl])
                        nc.vector.tensor_add(sg[:, 1, gsl], cT[0:HID, gsl],
                                             t1[:, gsl])

                    # mid-block prefetch of the next X block
                    if tl == 4 and t + 4 < Wsteps:
                        xnext = load_block(t + 4)
                        xnext_fresh = True

                    tsb_n = None if last else emit_tsb(t + 1)

                    # next step's xfT: transposes go behind hmm_A on PE; the
                    # psum->sbuf copy sits early in the DVE stream (it parks
                    # until the transposes land, while later DVE ops bypass).
                    if not last:
                        if tl == TB - 1:
                            xcur = xnext
                        pxf = emit_trans(xcur, (t + 1) % TB)

                    for gi in range(2):
                        gsl = GSL[gi]
                        pg = pgrp[gi]

                        # h-side accumulate onto the x-side psum
                        rg = RGS[gi]
                        for g in range(4):
                            sl = slice((g % 2) * rg, (g % 2) * rg + rg)
                            nc.tensor.matmul(
                                pg[:, g // 2, sl],
                                wt["WallT"][:, HID * g:HID * (g + 1)],
                                hT[:, gsl], start=False, stop=(g % 2 == 1),
                            )

                        # one sigmoid instruction for all 4 gates of group;
                        # psum gate order (f,i | ct,o) -> sg slots (0,2|3,5)
                        nc.scalar.activation(
                            sg[:, 0:6, gsl].rearrange(
                                "p (b r) c -> p b r c", b=2)[:, :, ::2, :],
                            pg[:, :, 0:2 * rg].rearrange(
                                "p b (s c) -> p b s c", c=rg
                            ),
                            AF.Sigmoid,
                        )

                        # state update: c' = f*cadj + i*ct via one paired
                        # multiply over slots (0,2)x(1,3) then one add
                        pair = sg[:, 0:4, gsl].rearrange(
                            "p (a b) c -> p a b c", b=2)
                        nc.vector.tensor_mul(pp[:, :, gsl], pair[:, :, 0, :],
                                             pair[:, :, 1, :])
                        nc.vector.tensor_add(cT[0:HID, gsl], pp[:, 0, gsl],
                                             pp[:, 1, gsl])

                        # next step's x-side matmuls reuse this group's freed
                        # pg banks right after its sigmoid; Wd(t+1) parks
                        # in the PE wait queue and fires the moment c'_B lands.
                        if not last:
                            emit_wd(pds[0], gi)
                            if gi == 1:
                                # the xfT copy sits AFTER c'_B in the DVE
                                # program so the B-tail never queues behind
                                # it; both groups' x-matmuls follow it
                                xfT = xf_pool.tile([XC, R], bf16, tag="xfT")
                                nc.vector.tensor_scalar_add(xfT[:], pxf[:],
                                                            0.0)
                                for gj in range(2):
                                    for g in range(4):
                                        emit_xmm(pgrp[gj], gj, g, xfT)

                    # ---- step tails: h' = o*tanh(c'). Both tanhs emit
                    # before both h' muls so h'_A sits after c'_B in the
                    # DVE program and cannot shadow group B's tail.
                    for gi in range(2):
                        gsl = GSL[gi]
                        nc.scalar.activation(tnc[:, gsl], cT[0:HID, gsl],
                                             AF.Tanh)
                    for gi in range(2):
                        gsl = GSL[gi]
                        nc.vector.tensor_mul(hT[0:HID, gsl], sg[:, 5, gsl],
                                             tnc[:, gsl])

                    # one-hot expansion for the prefetched block rides in the
                    # DVE lull at step tails (one triple per step)
                    if xnext_fresh and tl in (4, 5, 6):
                        emit_onehot(xnext, tl - 4)
                        if tl == 6:
                            xnext_fresh = False

                    tsb = tsb_n

                # ---- output linear: nodes = relu(lin @ h + lb)
                pl = pd_pool.tile([HID, 512], f32, tag="pd0", name="pl")
                nc.tensor.matmul(pl[:, 0:R], wt["linT"][:], hT[:],
                                 start=True, stop=True)
                nc.scalar.activation(nodesT[:], pl[:, 0:R], AF.Relu)

            # ---------------- GCN: two SAGE layers + output proj
            with (
                tc.tile_pool(name="gc", bufs=2) as gc,
                tc.tile_pool(name="gcs", bufs=1) as gcs,
                tc.tile_pool(name="gp", bufs=2, space=PSUM) as gp,
                tc.tile_pool(name="gp1", bufs=1, space=PSUM) as gp1,
            ):
                def mean_agg(srcT, hid):
                    """srcT: [hid, R] feature-major -> aggT [hid, R].

                    Batches 3 graphs per transpose/matmul using the
                    block-diagonal [120,120] mean matrix: [g0|g1|g2] @ M3 =
                    [g0@M | g1@M | g2@M].
                    """
                    aggT = gcs.tile([hid, R], f32r, tag=f"agg{hid}", name="aggT")
                    for b0, nb in [(0, 3), (3, 3), (6, 2)]:
                        rows = N * nb
                        cols = srcT[:, N * b0:N * (b0 + nb)]   # [hid, 120]
                        ptr = gp.tile([3 * N, 128], f32, tag="ptr")
                        nc.tensor.transpose(
                            r(ptr[0:rows, 0:hid]), cols, wt["ident"][:hid, :hid]
                        )
                        nbm = gc.tile([3 * N, 128], f32r, tag="nbm")
                        nc.any.tensor_copy(out=nbm[0:rows, 0:hid],
                                           in_=ptr[0:rows, 0:hid])
                        pa = gp.tile([128, 3 * N], f32, tag="pa")
                        nc.tensor.matmul(
                            pa[0:hid, 0:rows], nbm[0:rows, 0:hid],
                            wt["Mmat3"][0:rows, 0:rows],
                            start=True, stop=True,
                        )
                        nc.any.tensor_copy(
                            out=aggT[:, N * b0:N * (b0 + nb)],
                            in_=pa[0:hid, 0:rows]
                        )
                    return aggT

                agg1 = mean_agg(nodesT, HID)
                pg1 = gp1.tile([64, R], f32, tag="pg1")
                nc.tensor.matmul(pg1, wt["s1l"][:], agg1[:], start=True, stop=False)
                nc.tensor.matmul(pg1, wt["s1r"][:], nodesT[:], start=False, stop=True)
                g1T = gcs.tile([64, R], f32r, tag="g1T")
                nc.scalar.activation(g1T[:], pg1, AF.Relu, bias=wt["s1b"][:].bitcast(f32))

                agg2 = mean_agg(g1T, 64)
                pg2 = gp1.tile([32, R], f32, tag="pg2")
                nc.tensor.matmul(pg2, wt["s2l"][:], agg2[:], start=True, stop=False)
                nc.tensor.matmul(pg2, wt["s2r"][:], g1T[:], start=False, stop=True)
                g2T = gcs.tile([32, R], f32r, tag="g2T")
                nc.scalar.activation(g2T[:], pg2, AF.Relu, bias=wt["s2b"][:].bitcast(f32))

                po = gp1.tile([2, R], f32, tag="po")
                nc.tensor.matmul(po, wt["ow"][:], g2T[:], start=True, stop=True)
                oT = gcs.tile([2, R], f32, tag="oT")
                nc.scalar.activation(oT[:], po, AF.Relu, bias=wt["ob"][:].bitcast(f32))

                nc.sync.dma_start(
                    out=out_ext.rearrange("k b n -> k (b n)"), in_=oT[:]
                )

    nc.compile()
    return nc


# ---------------------------------------------------------------- execution
_CACHE = {}


def _get_module():
    if "nc" not in _CACHE:
        _CACHE["nc"] = build_module()
    return _CACHE["nc"]


def make_in_maps(inputs):
    f32 = np.float32
    import ml_dtypes
    bf = ml_dtypes.bfloat16
    X = np.ascontiguousarray(np.asarray(inputs["X"], f32).astype(bf))
    ts = np.asarray(inputs["ts_list"], f32)
    wts = _host_weights(inputs)
    in_maps = []
    for c in range(NCORES):
        tsl = ts[c * BL:(c + 1) * BL]                       # [BL, W, N]
        tsm1 = (tsl.transpose(1, 0, 2).reshape(W, R) - 1.0).astype(bf)
        tsm1_rep = np.ascontiguousarray(
            np.broadcast_to(tsm1[None], (HID, W, R))
        )
        m = {"X": X[c * BL:(c + 1) * BL], "tsm1": tsm1_rep}
        m.update(wts)
        in_maps.append(m)
    return in_maps


def kernel(**inputs) -> np.ndarray:
    from concourse.bass_utils import run_bass_kernel_spmd

    nc = _get_module()
    in_maps = make_in_maps(inputs)
    res = run_bass_kernel_spmd(nc, in_maps, list(range(NCORES)))
    outs = [
        np.transpose(res.results[c]["out"], (1, 2, 0)) for c in range(NCORES)
    ]
    return np.ascontiguousarray(np.concatenate(outs, axis=0).astype(np.float32))


# revision 49
# speedup vs baseline: 1.0059x; 1.0003x over previous
"""AgentImputer Trainium2 kernel.

Contract: kernel(**inputs) takes the FULL unsharded inputs (as produced by
reference.setup_inputs()) and returns the FULL output [64, 40, 2] float32.

Strategy: data-parallel over batch B=64 across 8 NeuronCores (8 batches /
core -> 320 folded LSTM rows per core). The 128-step TimeLSTM runs
feature-major ([hid, row] tiles); categorical embeddings fold into the
input matmul via one-hot rows; biases fold into matmuls via a constant-1
state row. The recurrent loop is software-pipelined as TWO independent
column groups (rows 0:160 / 160:320) so the serial h->gates->c->h chain of
one group overlaps engine work of the other. All elementwise state math is
bf16 (DVE 2x packed mode); sigmoid over all 4 gates of a group is a single
ACT instruction; (ts-1) is host-precomputed and DMA-broadcast per step; the
per-graph GCN (shared edge_index) is dense [40,40] mean-aggregation matmuls.
"""

import sys

import numpy as np

sys.path.insert(0, "/opt/trn_rl_repo")

# ---------------------------------------------------------------- constants
B, W, N, F_IN = 64, 128, 40, 66
HID = 100
NUM_CONT = 64
NCLS_POS, NCLS_TEAM = 16, 9
EMB_POS, EMB_TEAM = 4, 3
NCORES = 8
BL = B // NCORES          # 8 local batch elems per core
R = BL * N                # 320 rows per core; row j = 40*b_local + n
RGS = [160, 160]          # pipeline column groups
OH_P0 = 66                # one-hot pos cols [66:83)
OH_T0 = 83                # one-hot team cols [83:100) (entries 10..16 pad)
XC = 100                  # xs tile feature columns
G4 = 4 * HID
TB = 8                    # timestep block for X prefetch


# ---------------------------------------------------------------- host prep
def _host_weights(inputs):
    import ml_dtypes
    bf = ml_dtypes.bfloat16
    f32 = np.float32
    Uall_w = np.asarray(inputs["Uall_w"], f32)       # [400, 71]
    Uall_b = np.asarray(inputs["Uall_b"], f32)       # [400]
    Wall_w = np.asarray(inputs["Wall_w"], f32)       # [400, 100]
    Wall_b = np.asarray(inputs["Wall_b"], f32)       # [400]
    Wd_w = np.asarray(inputs["Wd_w"], f32)           # [100, 100]
    Wd_b = np.asarray(inputs["Wd_b"], f32)           # [100]
    lin_w = np.asarray(inputs["lin_w"], f32)         # [100, 100]
    lin_b = np.asarray(inputs["lin_b"], f32)         # [100]
    emb_pos = np.asarray(inputs["emb_pos"], f32)     # [16, 4]
    emb_team = np.asarray(inputs["emb_team"], f32)   # [9, 3]
    edge_index = np.asarray(inputs["edge_index"]).astype(np.int64)  # [2, E]

    # Input-side weights [100, 400]: rows 0:64 continuous features; rows
    # 64,65 (raw categorical codes riding along in the transposed tile) get
    # zero weights; rows 66:83 / 83:93 are one-hot rows with the embedding
    # tables pre-multiplied in (code 0 == missing -> zero row); 93:100 pad.
    WxT = np.zeros((XC, G4), f32)
    WxT[0:NUM_CONT] = Uall_w[:, 0:NUM_CONT].T
    pad_pos = np.vstack([np.zeros((1, EMB_POS), f32), emb_pos])    # [17, 4]
    pad_team = np.vstack([np.zeros((1, EMB_TEAM), f32), emb_team])  # [10, 3]
    WxT[OH_P0:OH_T0] = pad_pos @ Uall_w[:, NUM_CONT:NUM_CONT + EMB_POS].T
    WxT[OH_T0:OH_T0 + NCLS_TEAM + 1] = (
        pad_team @ Uall_w[:, NUM_CONT + EMB_POS:].T
    )

    # h-side weights with the full gate bias folded in as an extra row
    # (state tiles carry a constant-1 row at partition HID).
    WallT = np.concatenate([Wall_w.T, (Wall_b + Uall_b)[None, :]], 0)  # [101, 400]
    WdT = np.concatenate([Wd_w.T, Wd_b[None, :]], 0)                   # [101, 100]
    linT = np.concatenate([lin_w.T, lin_b[None, :]], 0)                # [101, 100]

    # Mean-aggregation matrix: M[s, d] = count(s->d) / max(deg(d), 1)
    src, dst = edge_index[0], edge_index[1]
    cnt = np.zeros((N, N), f32)
    np.add.at(cnt, (src, dst), 1.0)
    deg = np.maximum(cnt.sum(axis=0), 1.0)
    Mmat = cnt / deg[None, :]

    # iota rows for the merged one-hot compare: [0..16 | 0..9, -1 x7],
    # replicated for each timestep of an 8-step block
    iota2 = np.concatenate([
        np.arange(NCLS_POS + 1, dtype=f32),
        np.concatenate([np.arange(NCLS_TEAM + 1, dtype=f32),
                        -np.ones(17 - (NCLS_TEAM + 1), f32)]),
    ])
    iota2b = np.tile(iota2, TB)  # [TB*34]

    # gate-chunk order (f, i, ct, o): lets the kernel pair (f,i) with
    # (cadj,ct) in one strided DVE multiply
    gperm = np.concatenate([np.arange(100), np.arange(100, 200),
                            np.arange(300, 400), np.arange(200, 300)])
    WxT = WxT[:, gperm]
    WallT = WallT[:, gperm]

    return {
        "WxT": WxT.astype(bf),
        "WallT": WallT.astype(bf),
        "WdT": WdT.astype(bf),
        "linT": linT.astype(bf),
        "Mmat3": np.ascontiguousarray(
            np.kron(np.eye(3, dtype=f32), Mmat), f32),  # [120,120] blockdiag
        "s1l": np.ascontiguousarray(np.asarray(inputs["sage1_l"], f32).T),   # [100, 64]
        "s1r": np.ascontiguousarray(np.asarray(inputs["sage1_r"], f32).T),   # [100, 64]
        "s1b": np.ascontiguousarray(np.asarray(inputs["sage1_lb"], f32)[:, None]),  # [64, 1]
        "s2l": np.ascontiguousarray(np.asarray(inputs["sage2_l"], f32).T),   # [64, 32]
        "s2r": np.ascontiguousarray(np.asarray(inputs["sage2_r"], f32).T),   # [64, 32]
        "s2b": np.ascontiguousarray(np.asarray(inputs["sage2_lb"], f32)[:, None]),  # [32, 1]
        "ow": np.ascontiguousarray(np.asarray(inputs["out_w"], f32).T),      # [32, 2]
        "ob": np.ascontiguousarray(np.asarray(inputs["out_b"], f32)[:, None]),      # [2, 1]
        "iota2b": np.tile(iota2b, (120, 1)).astype(bf),                      # [120, TB*34]
        "hcinit": np.concatenate(
            [np.zeros((HID, R), f32), np.ones((1, R), f32)], 0
        ).astype(bf),  # [101, R]: zero state + constant-1 bias row
        "ident": np.eye(128, dtype=f32),
        "identb": np.eye(128, dtype=bf),
    }


# ---------------------------------------------------------------- device IR
def build_module(Wsteps=W):
    import concourse.bass as bass
    import concourse.tile as tile
    from concourse import bacc, mybir

    f32 = mybir.dt.float32
    f32r = mybir.dt.float32r
    bf16 = mybir.dt.bfloat16
    AF = mybir.ActivationFunctionType
    EQ = mybir.AluOpType.is_equal
    PSUM = bass.MemorySpace.PSUM

    def r(ap):
        return ap.bitcast(f32r)

    nc = bacc.Bacc(
        "TRN2", target_bir_lowering=False, debug=False, num_devices=NCORES
    )

    X_in = nc.declare_dram_parameter("X", [BL, W, N, F_IN], bf16, isOutput=False)
    # host-precomputed (ts-1), replicated across 100 partitions: [100, W, R]
    tsm1_in = nc.declare_dram_parameter("tsm1", [HID, W, R], bf16, isOutput=False)
    w_in = {}
    bf16_params = {"WxT", "WallT", "WdT", "linT", "iota2b", "identb", "hcinit"}
    for name, shape in [
        ("WxT", [XC, G4]), ("WallT", [HID + 1, G4]), ("WdT", [HID + 1, HID]),
        ("linT", [HID + 1, HID]), ("Mmat3", [3 * N, 3 * N]),
        ("s1l", [HID, 64]), ("s1r", [HID, 64]), ("s1b", [64, 1]),
        ("s2l", [64, 32]), ("s2r", [64, 32]), ("s2b", [32, 1]),
        ("ow", [32, 2]), ("ob", [2, 1]),
        ("iota2b", [120, TB * 34]), ("hcinit", [HID + 1, R]),
        ("ident", [128, 128]), ("identb", [128, 128]),
    ]:
        w_in[name] = nc.declare_dram_parameter(
            name, shape, bf16 if name in bf16_params else f32r, isOutput=False
        )
    # device-natural layout [k, b, n]; host transposes to [b, n, k]
    out_ext = nc.declare_dram_parameter("out", [2, BL, N], f32, isOutput=True)

    GSL = [slice(0, RGS[0]), slice(RGS[0], R)]

    with tile.TileContext(nc) as tc:
        with (
            tc.tile_pool(name="consts", bufs=1) as consts,
            tc.tile_pool(name="state", bufs=1) as state,
        ):
            # ---- load constants / weights
            wt = {}
            qs = [nc.gpsimd, nc.sync]
            for qi, (name, ext) in enumerate(w_in.items()):
                wt[name] = consts.tile(
                    list(ext.shape), ext.dtype, tag=name, name=name
                )
                qs[qi % 2].dma_start(out=wt[name][:], in_=ext[:])

            # ---- persistent state: h/c feature-major with const-1 bias row
            hT = state.tile([HID + 1, R], bf16, tag="hT")
            cT = state.tile([HID + 1, R], bf16, tag="cT")
            nc.gpsimd.dma_start(out=hT[:], in_=w_in["hcinit"][:])
            nc.gpsimd.dma_start(out=cT[:], in_=w_in["hcinit"][:])

            nodesT = state.tile([HID, R], f32r, tag="nodesT")

            Xnb = X_in.rearrange("b t n f -> b n t f")

            with (
                tc.tile_pool(name="xs", bufs=2) as xs_pool,
                tc.tile_pool(name="xf", bufs=2) as xf_pool,
                tc.tile_pool(name="tsb", bufs=2) as tsb_pool,
                tc.tile_pool(name="sg", bufs=2) as sg_pool,
                tc.tile_pool(name="work", bufs=2) as work,
                tc.tile_pool(name="pga", bufs=1, space=PSUM) as pga_pool,
                tc.tile_pool(name="pgb", bufs=1, space=PSUM) as pgb_pool,
                tc.tile_pool(name="pd", bufs=1, space=PSUM) as pd_pool,
                tc.tile_pool(name="pxf", bufs=2, space=PSUM) as pxf_pool,
            ):
                TRIPLES = [(0, 3), (3, 3), (6, 2)]

                def load_block(t0):
                    """DMA one TB-step X block (one-hots emitted separately)."""
                    tiles = []
                    for k, (b0, nb) in enumerate(TRIPLES):
                        xt = xs_pool.tile([120, TB, XC], bf16,
                                          tag=f"xs{k}", name=f"xs{k}")
                        for i in range(nb):
                            nc.sync.dma_start(
                                out=xt[N * i:N * (i + 1), :, 0:F_IN],
                                in_=Xnb[b0 + i, :, t0:t0 + TB, :],
                            )
                        tiles.append(xt)
                    return tiles

                def emit_onehot(xtiles, k):
                    # merged one-hot for triple k: both categorical cols,
                    # all TB steps, all stacked graphs in one op
                    rows = N * TRIPLES[k][1]
                    xt = xtiles[k]
                    nc.vector.tensor_tensor(
                        out=xt[:rows, :, OH_P0:XC].rearrange(
                            "p t (g k) -> p t g k", k=17
                        ),
                        in0=wt["iota2b"][0:rows, :].rearrange(
                            "p (t g k) -> p t g k", t=TB, k=17
                        ),
                        in1=xt[
                            :rows, :, NUM_CONT:NUM_CONT + 2
                        ].to_broadcast([rows, TB, 2, 17]),
                        op=EQ,
                    )

                def emit_trans(xtiles, tl):
                    """PE transposes -> pxf psum; returns psum tile."""
                    pxf = pxf_pool.tile([XC, R], bf16, tag="pxf")
                    for k, (b0, nb) in enumerate(TRIPLES):
                        rows = N * nb
                        nc.tensor.transpose(
                            pxf[:, 120 * k:120 * k + rows],
                            xtiles[k][:rows, tl, :],
                            wt["identb"][:rows, :rows],
                        )
                    return pxf

                def emit_tsb(t0):
                    # whole TB-step block of (ts-1) rows in one DMA
                    tsb = tsb_pool.tile([HID, TB, R], bf16, tag="tsb")
                    nc.sync.dma_start(out=tsb[:],
                                      in_=tsm1_in[:, t0:t0 + TB, :])
                    return tsb

                def emit_xmm(pg, gi, g, xfT):
                    rg = RGS[gi]
                    sl = slice((g % 2) * rg, (g % 2) * rg + rg)
                    nc.tensor.matmul(
                        pg[:, g // 2, sl],
                        wt["WxT"][:, HID * g:HID * (g + 1)],
                        xfT[:, GSL[gi]], start=(g % 2 == 0), stop=False,
                    )

                def emit_wd(pdn, gi):
                    # two half-width Wd matmuls share one psum bank: the A
                    # half's start arms the bank, B's half closes the group;
                    # each fires as soon as its own c' half lands.
                    nc.tensor.matmul(pdn[:, GSL[gi]], wt["WdT"][:],
                                     cT[:, GSL[gi]],
                                     start=(gi == 0), stop=(gi == 1))

                # ---- prologue: block 0. Steps 0-1 are DMA'd first as a thin
                # slice so compute starts while the rest of the block loads.
                xcur = []
                for k, (b0, nb) in enumerate(TRIPLES):
                    xt = xs_pool.tile([120, TB, XC], bf16,
                                      tag=f"xs{k}", name=f"xs{k}")
                    for i in range(nb):
                        nc.sync.dma_start(
                            out=xt[N * i:N * (i + 1), 0:2, 0:F_IN],
                            in_=Xnb[b0 + i, :, 0:2, :],
                        )
                    rows = N * nb
                    nc.vector.tensor_tensor(
                        out=xt[:rows, 0:2, OH_P0:XC].rearrange(
                            "p t (g k) -> p t g k", k=17
                        ),
                        in0=wt["iota2b"][0:rows, 0:2 * 34].rearrange(
                            "p (t g k) -> p t g k", t=2, k=17
                        ),
                        in1=xt[
                            :rows, 0:2, NUM_CONT:NUM_CONT + 2
                        ].to_broadcast([rows, 2, 2, 17]),
                        op=EQ,
                    )
                    xcur.append(xt)
                for k, (b0, nb) in enumerate(TRIPLES):
                    xt = xcur[k]
                    for i in range(nb):
                        nc.sync.dma_start(
                            out=xt[N * i:N * (i + 1), 2:TB, 0:F_IN],
                            in_=Xnb[b0 + i, :, 2:TB, :],
                        )
                    rows = N * nb
                    nc.vector.tensor_tensor(
                        out=xt[:rows, 2:TB, OH_P0:XC].rearrange(
                            "p t (g k) -> p t g k", k=17
                        ),
                        in0=wt["iota2b"][0:rows, 0:(TB - 2) * 34].rearrange(
                            "p (t g k) -> p t g k", t=TB - 2, k=17
                        ),
                        in1=xt[
                            :rows, 2:TB, NUM_CONT:NUM_CONT + 2
                        ].to_broadcast([rows, TB - 2, 2, 17]),
                        op=EQ,
                    )
                pxf0 = emit_trans(xcur, 0)
                xfT = xf_pool.tile([XC, R], bf16, tag="xfT")
                nc.vector.tensor_scalar_add(xfT[:], pxf0[:], 0.0)
                pgA = pga_pool.tile([HID, 2, 512], f32, tag="pgA", name="pgA")
                pgB = pgb_pool.tile([HID, 2, 512], f32, tag="pgB", name="pgB")
                pgrp = [pgA, pgB]
                for gi in range(2):
                    for g in range(4):
                        emit_xmm(pgrp[gi], gi, g, xfT)
                tsb = emit_tsb(0)
                tsb_n = None
                pds = [pd_pool.tile([HID, 512], f32, tag="pd0", name="pd0")]
                emit_wd(pds[0], 0)
                emit_wd(pds[0], 1)
                xnext_fresh = False

                for t in range(Wsteps):
                    tl = t % TB
                    last = t == Wsteps - 1

                    t1 = work.tile([HID, R], bf16, tag="t1")
                    cs1 = work.tile([HID, R], bf16, tag="cs1")
                    tnc = work.tile([HID, R], bf16, tag="tnc")
                    pp = work.tile([HID, 2, R], bf16, tag="pp")
                    # sg slots: 0=f, 1=cadj, 2=i, 3=ct, 4=unused, 5=o
                    sg = sg_pool.tile([HID, 6, R], bf16, tag="sg")

                    # ---- c path (off the critical h-chain); cadj lands in
                    # sg slot 1, adjacent to the gates. t1/cadj split per
                    # group, A first, so cadj_A is ready before sigA ends.
                    nc.scalar.activation(cs1[:], pds[0][:, 0:R], AF.Tanh)
                    for gi in range(2):
                        gsl = GSL[gi]
                        nc.vector.tensor_mul(t1[:, gsl], cs1[:, gsl],
                                             tsb[:, tl, gsl])
                        nc.vector.tensor_add(sg[:, 1, gsl], cT[0:HID, gsl],
                                             t1[:, gsl])

                    # mid-block prefetch of the next X block
                    if tl == 4 and t + 4 < Wsteps:
                        xnext = load_block(t + 4)
                        xnext_fresh = True

                    # prefetch next block's (ts-1) rows mid-block
                    if tl == 4 and t + 4 < Wsteps:
                        tsb_n = emit_tsb(t + 4)

                    # next step's xfT: transposes go behind hmm_A on PE; the
                    # psum->sbuf copy sits early in the DVE stream (it parks
                    # until the transposes land, while later DVE ops bypass).
                    if not last:
                        if tl == TB - 1:
                            xcur = xnext
                        pxf = emit_trans(xcur, (t + 1) % TB)

                    for gi in range(2):
                        gsl = GSL[gi]
                        pg = pgrp[gi]

                        # h-side accumulate onto the x-side psum
                        rg = RGS[gi]
                        for g in range(4):
                            sl = slice((g % 2) * rg, (g % 2) * rg + rg)
                            nc.tensor.matmul(
                                pg[:, g // 2, sl],
                                wt["WallT"][:, HID * g:HID * (g + 1)],
                                hT[:, gsl], start=False, stop=(g % 2 == 1),
                            )

                        # one sigmoid instruction for all 4 gates of group;
                        # psum gate order (f,i | ct,o) -> sg slots (0,2|3,5)
                        nc.scalar.activation(
                            sg[:, 0:6, gsl].rearrange(
                                "p (b r) c -> p b r c", b=2)[:, :, ::2, :],
                            pg[:, :, 0:2 * rg].rearrange(
                                "p b (s c) -> p b s c", c=rg
                            ),
                            AF.Sigmoid,
                        )

                        # state update: c' = f*cadj + i*ct via one paired
                        # multiply over slots (0,2)x(1,3) then one add
                        pair = sg[:, 0:4, gsl].rearrange(
                            "p (a b) c -> p a b c", b=2)
                        nc.vector.tensor_mul(pp[:, :, gsl], pair[:, :, 0, :],
                                             pair[:, :, 1, :])
                        nc.vector.tensor_add(cT[0:HID, gsl], pp[:, 0, gsl],
                                             pp[:, 1, gsl])

                        # next step's x-side matmuls reuse this group's freed
                        # pg banks right after its sigmoid; Wd(t+1) parks
                        # in the PE wait queue and fires the moment c'_B lands.
                        if not last:
                            emit_wd(pds[0], gi)
                            if gi == 1:
                                # the xfT copy sits AFTER c'_B in the DVE
                                # program so the B-tail never queues behind
                                # it; both groups' x-matmuls follow it
                                xfT = xf_pool.tile([XC, R], bf16, tag="xfT")
                                nc.vector.tensor_scalar_add(xfT[:], pxf[:],
                                                            0.0)
                                for gj in range(2):
                                    for g in range(4):
                                        emit_xmm(pgrp[gj], gj, g, xfT)

                    # ---- step tails: h' = o*tanh(c'). Both tanhs emit
                    # before both h' muls so h'_A sits after c'_B in the
                    # DVE program and cannot shadow group B's tail.
                    for gi in range(2):
                        gsl = GSL[gi]
                        nc.scalar.activation(tnc[:, gsl], cT[0:HID, gsl],
                                             AF.Tanh)
                    for gi in range(2):
                        gsl = GSL[gi]
                        nc.vector.tensor_mul(hT[0:HID, gsl], sg[:, 5, gsl],
                                             tnc[:, gsl])

                    # one-hot expansion for the prefetched block rides in the
                    # DVE lull at step tails (one triple per step)
                    if xnext_fresh and tl in (4, 5, 6):
                        emit_onehot(xnext, tl - 4)
                        if tl == 6:
                            xnext_fresh = False

                    if tl == TB - 1:
                        tsb = tsb_n

                # ---- output linear: nodes = relu(lin @ h + lb)
                pl = pd_pool.tile([HID, 512], f32, tag="pd0", name="pl")
                nc.tensor.matmul(pl[:, 0:R], wt["linT"][:], hT[:],
                                 start=True, stop=True)
                nc.scalar.activation(nodesT[:], pl[:, 0:R], AF.Relu)

            # ---------------- GCN: two SAGE layers + output proj
            with (
                tc.tile_pool(name="gc", bufs=2) as gc,
                tc.tile_pool(name="gcs", bufs=1) as gcs,
                tc.tile_pool(name="gp", bufs=2, space=PSUM) as gp,
                tc.tile_pool(name="gp1", bufs=1, space=PSUM) as gp1,
            ):
                def mean_agg(srcT, hid):
                    """srcT: [hid, R] feature-major -> aggT [hid, R].

                    Batches 3 graphs per transpose/matmul using the
                    block-diagonal [120,120] mean matrix: [g0|g1|g2] @ M3 =
                    [g0@M | g1@M | g2@M].
                    """
                    aggT = gcs.tile([hid, R], f32r, tag=f"agg{hid}", name="aggT")
                    for b0, nb in [(0, 3), (3, 3), (6, 2)]:
                        rows = N * nb
                        cols = srcT[:, N * b0:N * (b0 + nb)]   # [hid, 120]
                        ptr = gp.tile([3 * N, 128], f32, tag="ptr")
                        nc.tensor.transpose(
                            r(ptr[0:rows, 0:hid]), cols, wt["ident"][:hid, :hid]
                        )
                        nbm = gc.tile([3 * N, 128], f32r, tag="nbm")
                        nc.any.tensor_copy(out=nbm[0:rows, 0:hid],
                                           in_=ptr[0:rows, 0:hid])
                        pa = gp.tile([128, 3 * N], f32, tag="pa")
                        nc.tensor.matmul(
                            pa[0:hid, 0:rows], nbm[0:rows, 0:hid],
                            wt["Mmat3"][0:rows, 0:rows],
                            start=True, stop=True,
                        )
                        nc.any.tensor_copy(
                            out=aggT[:, N * b0:N * (b0 + nb)],
                            in_=pa[0:hid, 0:rows]
                        )
                    return aggT

                agg1 = mean_agg(nodesT, HID)
                pg1 = gp1.tile([64, R], f32, tag="pg1")
                nc.tensor.matmul(pg1, wt["s1l"][:], agg1[:], start=True, stop=False)
                nc.tensor.matmul(pg1, wt["s1r"][:], nodesT[:], start=False, stop=True)
                g1T = gcs.tile([64, R], f32r, tag="g1T")
                nc.scalar.activation(g1T[:], pg1, AF.Relu, bias=wt["s1b"][:].bitcast(f32))

                agg2 = mean_agg(g1T, 64)
                pg2 = gp1.tile([32, R], f32, tag="pg2")
                nc.tensor.matmul(pg2, wt["s2l"][:], agg2[:], start=True, stop=False)
                nc.tensor.matmul(pg2, wt["s2r"][:], g1T[:], start=False, stop=True)
                g2T = gcs.tile([32, R], f32r, tag="g2T")
                nc.scalar.activation(g2T[:], pg2, AF.Relu, bias=wt["s2b"][:].bitcast(f32))

                po = gp1.tile([2, R], f32, tag="po")
                nc.tensor.matmul(po, wt["ow"][:], g2T[:], start=True, stop=True)
                oT = gcs.tile([2, R], f32, tag="oT")
                nc.scalar.activation(oT[:], po, AF.Relu, bias=wt["ob"][:].bitcast(f32))

                nc.sync.dma_start(
                    out=out_ext.rearrange("k b n -> k (b n)"), in_=oT[:]
                )

    nc.compile()
    return nc


# ---------------------------------------------------------------- execution
_CACHE = {}


def _get_module():
    if "nc" not in _CACHE:
        _CACHE["nc"] = build_module()
    return _CACHE["nc"]


def make_in_maps(inputs):
    f32 = np.float32
    import ml_dtypes
    bf = ml_dtypes.bfloat16
    X = np.ascontiguousarray(np.asarray(inputs["X"], f32).astype(bf))
    ts = np.asarray(inputs["ts_list"], f32)
    wts = _host_weights(inputs)
    in_maps = []
    for c in range(NCORES):
        tsl = ts[c * BL:(c + 1) * BL]                       # [BL, W, N]
        tsm1 = (tsl.transpose(1, 0, 2).reshape(W, R) - 1.0).astype(bf)
        tsm1_rep = np.ascontiguousarray(
            np.broadcast_to(tsm1[None], (HID, W, R))
        )
        m = {"X": X[c * BL:(c + 1) * BL], "tsm1": tsm1_rep}
        m.update(wts)
        in_maps.append(m)
    return in_maps


def kernel(**inputs) -> np.ndarray:
    from concourse.bass_utils import run_bass_kernel_spmd

    nc = _get_module()
    in_maps = make_in_maps(inputs)
    res = run_bass_kernel_spmd(nc, in_maps, list(range(NCORES)))
    outs = [
        np.transpose(res.results[c]["out"], (1, 2, 0)) for c in range(NCORES)
    ]
    return np.ascontiguousarray(np.concatenate(outs, axis=0).astype(np.float32))
